# revision 12
# baseline (speedup 1.0000x reference)
"""Bass/Trainium2 kernel for nn_LogReg_8151847928094.

out[b] = sum_s w[text[s, b]] + bias   (bag-of-words logistic regression)

Strategy (8 NeuronCores, batch-sharded 2048 -> 8 x 256 columns):
  - The [B, V] counts matrix is never built: out[b] is a sum of gathered
    w values.  The gather runs on-device via gpsimd.local_scatter, which
    (unlike ap_gather) takes PER-PARTITION independent indices and runs
    out of fast local Q7 RAM: data[p, j] -> dst[p, idxs[p, j]], dst
    zeroed first, -1 indices skipped (~1.4us per 784-long stream vs
    ~29ns/index for ap_gather's request-bound SBUF walk).
  - The 8 cores share one HBM, so the steady-state cost is DMA bytes.
    Index streams are sent as uint8: code 0 = skip, code k = dst
    position k-1 (max 254).  Each call c therefore serves only 255 of
    the 256 columns via position p = (col - c) mod 256; the omitted
    column (255 + c) mod 256 rotates, and the psum of each call is
    added into the accumulator at a shifted offset to undo the mapping.
  - Vocab layout: two fixed pseudorandom permutations sigma_{A,B}
    (text-independent), alternating per call: t' = sigma[t], partition
    p = t' % 128, data index h = t' // 128 < 784.  data[p, h] = w[t] in
    bf16.  Alternating permutations disperse (partition, column)
    pile-ups (the jax inputs have columns where many tokens share
    t mod 32) so one dst slot per column per call suffices.
  - Host greedily assigns every token to a call: at most one token per
    (call, p, h) and per (call, p, column).  6 calls place all 51200
    tokens per core (floor: max token multiplicity per core).
  - Per call: DVE widens uint8 codes to int16 (subtract 1), Pool
    scatters, PE ones-matmul reduces partitions into its own psum, DVE
    adds the shifted psum into the accumulator (initialized with the
    bias).  DMAs alternate between the SP and ACT queues.
"""

import sys

sys.path.insert(0, "/opt/trn_rl_repo")

import numpy as np
import ml_dtypes

import concourse.bass as bass
import concourse.bacc as bacc
import concourse.mybir as mybir
import concourse.tile as tile
from concourse.bass_utils import run_bass_kernel_spmd

S = 200
B = 2048
V = 100000
NCORES = 8
BS = B // NCORES  # 256 batch columns per core
P = 128
HIW = 784  # padded slice width (hi < 782)
NSIG = 2

# Up to this many local_scatter calls; the assignment typically uses 6.
MAX_CALLS = 12

# Fixed pseudorandom vocab permutations (text-independent).
SIGMAS = [np.random.default_rng(1000 + c).permutation(V) for c in range(NSIG)]

_prog_cache = {}


def _build_program(ncalls, loop_T=None):
    nc = bacc.Bacc("TRN2", target_bir_lowering=False, debug=False)
    data_ds = [
        nc.declare_dram_parameter(f"data{q}", [P, HIW], mybir.dt.bfloat16, isOutput=False)
        for q in range(NSIG)
    ]
    idx_d = nc.declare_dram_parameter(
        "idxall", [ncalls, P, HIW], mybir.dt.uint8, isOutput=False
    )
    bias_d = nc.declare_dram_parameter("bias", [1, BS], mybir.dt.float32, isOutput=False)
    out_d = nc.declare_dram_parameter("out", [1, BS], mybir.dt.float32, isOutput=True)

    nsets = 1 if loop_T is None else 2
    with tile.TileContext(nc) as tc:
        with (
            tc.tile_pool(name="sbuf", bufs=1) as pool,
            tc.tile_pool(name="psum", bufs=1, space="PSUM") as psum_pool,
        ):
            ones_t = pool.tile([P, 1], mybir.dt.bfloat16)
            sets = []
            for s in range(nsets):
                data_ts = [
                    pool.tile([P, HIW], mybir.dt.bfloat16, name=f"data{s}_{q}")
                    for q in range(NSIG)
                ]
                u8_t = pool.tile([P, ncalls * HIW], mybir.dt.uint8, name=f"u8_{s}")
                idx_t = pool.tile([P, ncalls * HIW], mybir.dt.int16, name=f"idx{s}")
                dst_ts = [
                    pool.tile([P, BS], mybir.dt.bfloat16, name=f"dst{s}_{c}")
                    for c in range(ncalls)
                ]
                psum_ts = [
                    psum_pool.tile([1, BS], mybir.dt.float32, name=f"psum{s}_{c}")
                    for c in range(2)
                ]
                bias_t = pool.tile([1, BS], mybir.dt.float32, name=f"bias{s}")
                acc_t = pool.tile([1, BS], mybir.dt.float32, name=f"acc{s}")
                sets.append((data_ts, u8_t, idx_t, dst_ts, psum_ts, bias_t, acc_t))

            nc.vector.memset(ones_t[:], 1.0)
            # SP and ACT are the available HWDGE queues; alternate them.
            dma_engs = [nc.sync, nc.scalar]

            def body(s):
                data_ts, u8_t, idx_t, dst_ts, psum_ts, bias_t, acc_t = sets[s]
                nc.sync.dma_start(
                    out=u8_t[:].rearrange("p (c j) -> p c j", c=ncalls),
                    in_=idx_d[:].rearrange("c p j -> p c j"),
                )
                for q in range(NSIG):
                    dma_engs[q % 2].dma_start(out=data_ts[q][:], in_=data_ds[q][:])
                nc.scalar.dma_start(out=bias_t[:], in_=bias_d[:])
                # Single widen on the otherwise-idle ACT engine; DVE keeps
                # the psum recombination.
                nc.scalar.activation(
                    idx_t[:],
                    u8_t[:],
                    mybir.ActivationFunctionType.Copy,
                    bias=-1.0,
                )
                for c in range(ncalls):
                    nc.gpsimd.local_scatter(
                        dst_ts[c][:],
                        data_ts[c % NSIG][:],
                        idx_t[:, c * HIW : (c + 1) * HIW],
                        channels=P,
                        num_elems=BS,
                        num_idxs=HIW,
                    )
                # acc = bias + sum of per-call psums, each shifted so dst
                # position p contributes to column (p + c) mod 256.  psum
                # tiles are double-buffered (PSUM has 8 banks total).
                nc.vector.tensor_scalar_add(acc_t[:], bias_t[:], 0.0)
                for c in range(ncalls):
                    pt = psum_ts[c % 2]
                    nc.tensor.matmul(
                        pt[:, 0 : BS - 1],
                        lhsT=ones_t[:],
                        rhs=dst_ts[c][:, 0 : BS - 1],
                        start=True,
                        stop=True,
                    )
                    sh = c % BS
                    hi_n = BS - sh  # positions 0 .. hi_n-1 -> cols sh .. 255
                    hi_n = min(hi_n, BS - 1)
                    nc.vector.tensor_tensor(
                        out=acc_t[:, sh : sh + hi_n],
                        in0=acc_t[:, sh : sh + hi_n],
                        in1=pt[:, 0:hi_n],
                        op=mybir.AluOpType.add,
                    )
                    if sh >= 2:
                        # positions 256-sh .. 254 -> cols 0 .. sh-2
                        nc.vector.tensor_tensor(
                            out=acc_t[:, 0 : sh - 1],
                            in0=acc_t[:, 0 : sh - 1],
                            in1=pt[:, BS - sh : BS - 1],
                            op=mybir.AluOpType.add,
                        )
                nc.sync.dma_start(out=out_d[:], in_=acc_t[:])

            if loop_T is None:
                body(0)
            else:
                # Two independent buffer sets per trip so consecutive
                # iterations pipeline (per-trip time = 2 executions).
                with tc.For_i(0, loop_T, 1) as _i:
                    body(0)
                    body(1)
    nc.compile()
    return nc


def _rank_within(key):
    """rank of each element among equal keys (stable order)."""
    order = np.argsort(key, kind="stable")
    ks = key[order]
    newrun = np.r_[True, ks[1:] != ks[:-1]]
    starts = np.flatnonzero(newrun)
    runid = np.cumsum(newrun) - 1
    rank = np.empty(key.size, np.int64)
    rank[order] = np.arange(key.size) - starts[runid]
    return rank


def _assign_core(text_block):
    """Greedy per-call assignment. Returns list of uint8 code maps.

    Per call, up to 3 mini-rounds re-offer tokens that lost one rank
    lottery while the other resource stayed free.
    """
    traw = text_block.astype(np.int64).ravel(order="F")
    b = np.repeat(np.arange(BS, dtype=np.int64), S)
    unassigned = np.ones(traw.size, bool)
    out = []
    for call in range(MAX_CALLS):
        if not unassigned.any():
            break
        t = SIGMAS[call % NSIG][traw]
        lo = t % P
        hi = t // P
        entry = lo * HIW + hi
        colk = lo * BS + b
        omit = (BS - 1 + call) % BS
        entryused = np.zeros(P * HIW, bool)
        colused = np.zeros(P * BS, bool)
        codes = np.zeros((P, HIW), np.uint8)
        for _mr in range(3):
            idx = np.flatnonzero(unassigned)
            if idx.size == 0:
                break
            ok = ~entryused[entry[idx]] & ~colused[colk[idx]] & (b[idx] != omit)
            cand = idx[ok]
            if cand.size == 0:
                break
            r1 = _rank_within(entry[cand])
            r2 = _rank_within(colk[cand])
            sel = (r1 < 1) & (r2 < 1)
            g = cand[sel]
            if g.size == 0:
                break
            pos = (b[g] - call) % BS  # <= 254 since b != omit
            codes[lo[g], hi[g]] = (pos + 1).astype(np.uint8)
            entryused[entry[g]] = True
            colused[colk[g]] = True
            unassigned[g] = False
        out.append(codes)
    assert not unassigned.any(), f"{unassigned.sum()} tokens unplaced"
    return out


def _prep_inputs(text, w, b):
    """Host prep shared by kernel() and the timing harness."""
    per_core = [_assign_core(text[:, c * BS : (c + 1) * BS]) for c in range(NCORES)]
    ncalls = max(len(a) for a in per_core)

    data_nps = []
    for q in range(NSIG):
        w_pad = np.zeros(HIW * P, np.float32)
        w_pad[SIGMAS[q]] = w
        data_nps.append(
            np.ascontiguousarray(w_pad.reshape(HIW, P).T).astype(ml_dtypes.bfloat16)
        )
    bias_row = np.full((1, BS), b[0], np.float32)

    empty = np.zeros((P, HIW), np.uint8)
    in_maps = []
    for c in range(NCORES):
        im = {"bias": bias_row}
        for q in range(NSIG):
            im[f"data{q}"] = data_nps[q]
        im["idxall"] = np.stack(
            [
                per_core[c][ci] if ci < len(per_core[c]) else empty
                for ci in range(ncalls)
            ]
        )
        in_maps.append(im)
    return ncalls, in_maps


def kernel(text, w, b):
    text = np.asarray(text)
    w = np.asarray(w, dtype=np.float32).reshape(-1)
    b = np.asarray(b, dtype=np.float32).reshape(-1)

    ncalls, in_maps = _prep_inputs(text, w, b)
    nc = _prog_cache.get(ncalls)
    if nc is None:
        nc = _build_program(ncalls)
        _prog_cache[ncalls] = nc

    res = run_bass_kernel_spmd(nc, in_maps, list(range(NCORES))).results
    out = np.concatenate([res[c]["out"][0] for c in range(NCORES)])
    return out.astype(np.float32)


if __name__ == "__main__":
    rng = np.random.default_rng(0)
    text = rng.integers(0, V, (S, B)).astype(np.int64)
    w = rng.standard_normal((1, V)).astype(np.float32) * 0.01
    b = np.zeros((1,), np.float32)
    out = kernel(text, w, b)
    exp = w[0][text].sum(axis=0) + b[0]
    err = np.abs(out - exp).max() / (np.abs(exp).max() + 1e-9)
    print("rel err:", err)
